# revision 6
# baseline (speedup 1.0000x reference)
"""Trainium2 Bass kernel for nn_Jointer: per-sample masked cosine-similarity.

out[b] = relu(l2norm(source[b]) @ l2norm(target[b]).T) * (mask_src[b] outer mask_tar[b])

Sharding: data-parallel over batch B=8 -> one sample per NeuronCore.
Per core: fp32 norms, PE-transpose both operands to [D, tokens] bf16,
bf16 matmul, fused scale+relu out of PSUM to fp16, row-pair (512KB)
output DMAs. Inputs split across both HWDGE rings (sync + scalar),
ACT tables primed at t=0, identity loaded as a constant input.
"""

import numpy as np

import concourse.bass as bass
from concourse import bacc
import concourse.mybir as mybir
import concourse.tile as tile
from concourse.bass_utils import run_bass_kernel_spmd

F32 = mybir.dt.float32
BF16 = mybir.dt.bfloat16
F16 = mybir.dt.float16
AF = mybir.ActivationFunctionType
ALU = mybir.AluOpType

S = 2048  # source tokens per sample
T = 2048  # target tokens per sample
D = 128  # feature dim (= contraction dim = partitions)
P = 128  # partitions
SB = S // P  # 16 source token blocks
TB = T // P  # 16 target token blocks
NT = 512  # matmul moving free dim (one PSUM bank of fp32)
G = 4  # blocks per transpose group
HB = TB // 2  # 8 blocks per half-side


def build_nc() -> bass.Bass:
    nc = bacc.Bacc(trn_type="TRN2")

    src = nc.dram_tensor("src", [S, D], F32, kind="ExternalInput")
    tgt = nc.dram_tensor("tgt", [T, D], F32, kind="ExternalInput")
    # maskf[p, k]: k in [0,16) source-block masks (token p*16+k),
    # k in [16,32) target-block masks (token (k-16)*128+p).
    maskf = nc.dram_tensor("maskf", [P, SB + TB], F32, kind="ExternalInput")
    identd = nc.dram_tensor("identd", [P, P], F32, kind="ExternalInput")
    out = nc.dram_tensor("out", [S, T], F16, kind="ExternalOutput")

    # source tokens in (p k) order: token p*16+k -> partition p, block k.
    # Per-partition DRAM lines are 16*128*4B = 8KB contiguous.
    src_r = src.rearrange("(p k) d -> p k d", p=P)
    # target tokens in (k p) order: token k*128+p -> partition p, block k,
    # so transposed tT columns are in natural token order.
    tgt_r = tgt.rearrange("(k p) d -> p k d", p=P)
    # out rows paired: row-pair q covers rows {p*16 + 2q + j, j in 0..1}.
    out_q = out.rearrange("(p q j) n -> q p j n", q=SB // 2, j=2)

    with tile.TileContext(nc) as tc:
        with (
            tc.tile_pool(name="singles", bufs=1) as singles,
            tc.tile_pool(name="inbuf", bufs=1) as inbuf,
            tc.tile_pool(name="norm", bufs=1) as normp,
            tc.tile_pool(name="pst", bufs=2, space="PSUM") as psum_t,
            tc.tile_pool(name="psmm", bufs=3, space="PSUM") as psum_mm,
            tc.tile_pool(name="outp", bufs=6) as outp,
        ):
            # --- prime the ACT function tables while DMAs are in flight.
            dummy = singles.tile([P, 4], F32)
            nc.vector.memset(dummy, 0)
            nc.scalar.activation(out=dummy, in_=dummy, func=AF.Square)
            nc.scalar.activation(out=dummy, in_=dummy, func=AF.Sqrt)

            ident = singles.tile([P, P], F32)
            mask_sb = singles.tile([P, SB + TB], F32)

            s_nat = inbuf.tile([P, SB, D], F32)
            sT = inbuf.tile([P, S], BF16)  # [D, s tokens] (raw, bf16)
            s_scl = normp.tile([P, SB], F32)  # rsqrt(|s|^2)*mask per token
            s_sq = inbuf.tile([P, SB, D], F32)
            t_nat = inbuf.tile([P, TB, D], F32)
            t_sc = inbuf.tile([P, TB, D], F32)  # normalized+masked target
            t_sq = inbuf.tile([P, TB, D], F32)
            t_ss = normp.tile([P, TB], F32)
            tT = inbuf.tile([P, T], BF16)  # [D, t tokens] normalized+masked

            # --- input DMAs: t side on the sync ring, s side + ident + mask
            # on the scalar ring, so issue and completion are parallel.
            nc.sync.dma_start(out=t_nat[:, 0:HB, :], in_=tgt_r[:, 0:HB, :])
            nc.scalar.dma_start(out=ident, in_=identd.rearrange("p q -> p q"))
            nc.scalar.dma_start(out=s_nat[:, 0:HB, :], in_=src_r[:, 0:HB, :])
            nc.sync.dma_start(out=t_nat[:, HB:TB, :], in_=tgt_r[:, HB:TB, :])
            nc.scalar.dma_start(out=mask_sb, in_=maskf.rearrange("p k -> p k"))
            nc.scalar.dma_start(out=s_nat[:, HB:SB, :], in_=src_r[:, HB:SB, :])

            cast_idx = [0]

            def xpose(src_tile, dstT, g, nm):
                # 4 PE transposes of fp32 [P,P] blocks -> one PSUM bank,
                # then one copy downconverting to bf16 (alternating engine).
                ps = psum_t.tile([P, G * P], F32, tag="pst", name=f"ps_{nm}{g}")
                for j in range(G):
                    k = g * G + j
                    nc.tensor.transpose(
                        ps[:, j * P : (j + 1) * P], src_tile[:, k, :], ident
                    )
                dst = dstT[:, g * G * P : (g + 1) * G * P]
                if cast_idx[0] % 2 == 0:
                    nc.vector.tensor_copy(out=dst, in_=ps)
                else:
                    nc.scalar.copy(out=dst, in_=ps)
                cast_idx[0] += 1

            def t_sqred(g):
                # square+reduce for one 4-block group (pipelines ACT/DVE).
                blk = slice(g * G, (g + 1) * G)
                nc.scalar.activation(
                    out=t_sq[:, blk, :], in_=t_nat[:, blk, :], func=AF.Square
                )
                nc.vector.reduce_sum(
                    out=t_ss[:, blk], in_=t_sq[:, blk, :], axis=mybir.AxisListType.X
                )

            def t_finish(h):
                # rsqrt + mask + broadcast prescale for half-side h.
                blk = slice(h * HB, (h + 1) * HB)
                t_rcp = normp.tile([P, HB], F32, tag="trcp", name=f"trcp{h}")
                nc.vector.reciprocal(out=t_rcp, in_=t_ss[:, blk])
                t_rsq = normp.tile([P, HB], F32, tag="trsq", name=f"trsq{h}")
                nc.scalar.activation(out=t_rsq, in_=t_rcp, func=AF.Sqrt)
                t_scl = normp.tile([P, HB], F32, tag="tscl2", name=f"tscl2_{h}")
                nc.vector.tensor_mul(
                    out=t_scl,
                    in0=t_rsq,
                    in1=mask_sb[:, SB + h * HB : SB + (h + 1) * HB],
                )
                scl_b = t_scl.unsqueeze(2).broadcast_to([P, HB, D])
                nc.vector.tensor_mul(
                    out=t_sc[:, blk, :], in0=t_nat[:, blk, :], in1=scl_b
                )

            def s_norm(h):
                # batched norm chain for half-side h (8 blocks).
                blk = slice(h * HB, (h + 1) * HB)
                nc.scalar.activation(
                    out=s_sq[:, blk, :], in_=s_nat[:, blk, :], func=AF.Square
                )
                s_ss = normp.tile([P, HB], F32, tag="sss", name=f"sss{h}")
                nc.vector.reduce_sum(
                    out=s_ss, in_=s_sq[:, blk, :], axis=mybir.AxisListType.X
                )
                s_rcp = normp.tile([P, HB], F32, tag="srcp", name=f"srcp{h}")
                nc.vector.reciprocal(out=s_rcp, in_=s_ss)
                s_rsq = normp.tile([P, HB], F32, tag="srsq", name=f"srsq{h}")
                nc.scalar.activation(out=s_rsq, in_=s_rcp, func=AF.Sqrt)
                nc.vector.tensor_mul(
                    out=s_scl[:, blk],
                    in0=s_rsq,
                    in1=mask_sb[:, h * HB : (h + 1) * HB],
                )

            # --- main: 2 MMs -> 1024-wide fused scale+relu copy -> fp16.
            # Row-pairs (2q, 2q+1) share one 512KB DMA; the two copies of a
            # pair go to different engines so they run concurrently.
            copy_idx = [0]

            def half_row(m, h, ob_j):
                ps = psum_mm.tile([P, 2 * NT], F32, tag="psmm", name=f"mm{m}_{h}")
                for qq in range(2):
                    n = 2 * h + qq
                    nc.tensor.matmul(
                        ps[:, qq * NT : (qq + 1) * NT],
                        sT[:, m * P : (m + 1) * P],
                        tT[:, n * NT : (n + 1) * NT],
                        start=True,
                        stop=True,
                    )
                i = copy_idx[0]
                copy_idx[0] += 1
                if i % 2 == 0:
                    nc.scalar.activation(
                        out=ob_j, in_=ps, func=AF.Relu, scale=s_scl[:, m : m + 1]
                    )
                else:
                    nc.vector.tensor_scalar(
                        out=ob_j,
                        in0=ps,
                        scalar1=s_scl[:, m : m + 1],
                        scalar2=0.0,
                        op0=ALU.mult,
                        op1=ALU.max,
                    )

            def pair(q, h):
                ob = outp.tile([P, 2, 2 * NT], F16, tag="ob", name=f"ob{q}_{h}")
                half_row(2 * q, h, ob[:, 0, :])
                half_row(2 * q + 1, h, ob[:, 1, :])
                nc.sync.dma_start(
                    out=out_q[q][:, :, h * 2 * NT : (h + 1) * 2 * NT], in_=ob
                )

            # Emission order == per-engine FIFO order. t half 0 is the
            # critical path to the first output pairs; s transposes start as
            # soon as s half 0 lands (no norm dependency); s norms must
            # complete just before the first out-copies.
            t_sqred(0)
            t_sqred(1)
            xpose(s_nat, sT, 0, "s")
            xpose(s_nat, sT, 1, "s")
            t_finish(0)
            s_norm(0)
            xpose(t_sc, tT, 0, "t")
            xpose(t_sc, tT, 1, "t")
            pair(0, 0)
            t_sqred(2)
            pair(1, 0)
            t_sqred(3)
            s_norm(1)
            pair(2, 0)
            xpose(s_nat, sT, 2, "s")
            pair(3, 0)
            xpose(s_nat, sT, 3, "s")
            pair(4, 0)
            t_finish(1)
            pair(5, 0)
            xpose(t_sc, tT, 2, "t")
            pair(6, 0)
            xpose(t_sc, tT, 3, "t")
            pair(7, 0)
            for q in range(SB // 2):
                pair(q, 1)

    nc.compile()
    return nc


_NC_CACHE = None


def _get_nc():
    global _NC_CACHE
    if _NC_CACHE is None:
        _NC_CACHE = build_nc()
    return _NC_CACHE


_IDENT = np.eye(P, dtype=np.float32)


def kernel(source, target, mask_src, mask_tar, **run_kwargs):
    source = np.asarray(source, dtype=np.float32)
    target = np.asarray(target, dtype=np.float32)
    mask_src = np.asarray(mask_src)
    mask_tar = np.asarray(mask_tar)
    B = source.shape[0]

    in_maps = []
    for b in range(B):
        # source tokens in (p k) order; target tokens in (k p) order.
        msf = mask_src[b].astype(np.float32).reshape(P, SB)
        mtf = mask_tar[b].astype(np.float32).reshape(TB, P).T
        mk = np.ascontiguousarray(np.concatenate([msf, mtf], axis=1))
        in_maps.append(
            {
                "src": np.ascontiguousarray(source[b]),
                "tgt": np.ascontiguousarray(target[b]),
                "maskf": mk,
                "identd": _IDENT,
            }
        )

    nc = _get_nc()
    res = run_bass_kernel_spmd(nc, in_maps, core_ids=list(range(B)), **run_kwargs)
    out = np.stack(
        [np.asarray(r["out"], dtype=np.float32) for r in res.results], axis=0
    )
    if run_kwargs.get("trace"):
        kernel.last_results = res
    return out


# revision 8
# speedup vs baseline: 1.1690x; 1.1690x over previous
"""Trainium2 Bass kernel for nn_Jointer: per-sample masked cosine-similarity.

out[b] = relu(l2norm(source[b]) @ l2norm(target[b]).T) * (mask_src[b] outer mask_tar[b])

Sharding: data-parallel over batch B=8 -> one sample per NeuronCore.
Per core: fp32 norms, PE-transpose both operands to [D, tokens] bf16,
bf16 matmul, fused scale+relu out of PSUM to fp16, row-pair (512KB)
output DMAs. All input DMAs on the sync ring ordered by criticality;
the target-side chain for the first output half runs group-at-a-time
with fused square+reduce on DVE, pinned via high_priority.
"""

import numpy as np

import concourse.bass as bass
from concourse import bacc
import concourse.mybir as mybir
import concourse.tile as tile
from concourse.bass_utils import run_bass_kernel_spmd

F32 = mybir.dt.float32
BF16 = mybir.dt.bfloat16
F16 = mybir.dt.float16
AF = mybir.ActivationFunctionType
ALU = mybir.AluOpType

S = 2048  # source tokens per sample
T = 2048  # target tokens per sample
D = 128  # feature dim (= contraction dim = partitions)
P = 128  # partitions
SB = S // P  # 16 source token blocks
TB = T // P  # 16 target token blocks
NT = 512  # matmul moving free dim (one PSUM bank of fp32)
G = 4  # blocks per transpose group
HB = TB // 2  # 8 blocks per half-side


def build_nc() -> bass.Bass:
    nc = bacc.Bacc(trn_type="TRN2")

    src = nc.dram_tensor("src", [S, D], F32, kind="ExternalInput")
    tgt = nc.dram_tensor("tgt", [T, D], F32, kind="ExternalInput")
    # maskf[p, k]: k in [0,16) source-block masks (token p*16+k),
    # k in [16,32) target-block masks (token (k-16)*128+p).
    maskf = nc.dram_tensor("maskf", [P, SB + TB], F32, kind="ExternalInput")
    identd = nc.dram_tensor("identd", [P, P], F32, kind="ExternalInput")
    out = nc.dram_tensor("out", [S, T], F16, kind="ExternalOutput")

    # source tokens in (p k) order: token p*16+k -> partition p, block k.
    # Per-partition DRAM lines are 16*128*4B = 8KB contiguous.
    src_r = src.rearrange("(p k) d -> p k d", p=P)
    # target tokens in (k p) order: token k*128+p -> partition p, block k,
    # so transposed tT columns are in natural token order.
    tgt_r = tgt.rearrange("(k p) d -> p k d", p=P)
    # out rows paired: row-pair q covers rows {p*16 + 2q + j, j in 0..1}.
    out_q = out.rearrange("(p q j) n -> q p j n", q=SB // 2, j=2)

    with tile.TileContext(nc) as tc:
        with (
            tc.tile_pool(name="singles", bufs=1) as singles,
            tc.tile_pool(name="inbuf", bufs=1) as inbuf,
            tc.tile_pool(name="norm", bufs=1) as normp,
            tc.tile_pool(name="pst", bufs=2, space="PSUM") as psum_t,
            tc.tile_pool(name="psmm", bufs=3, space="PSUM") as psum_mm,
            tc.tile_pool(name="outp", bufs=6) as outp,
        ):
            # --- prime the ACT function tables while DMAs are in flight.
            dummy = singles.tile([P, 4], F32)
            nc.vector.memset(dummy, 0)
            nc.scalar.activation(out=dummy, in_=dummy, func=AF.Square)
            nc.scalar.activation(out=dummy, in_=dummy, func=AF.Sqrt)

            ident = singles.tile([P, P], F32)
            mask_sb = singles.tile([P, SB + TB], F32)

            s_nat = inbuf.tile([P, SB, D], F32)
            sT = inbuf.tile([P, S], BF16)  # [D, s tokens] (raw, bf16)
            s_scl = normp.tile([P, SB], F32)  # rsqrt(|s|^2)*mask per token
            s_sq = inbuf.tile([P, SB, D], F32)
            t_nat = inbuf.tile([P, TB, D], F32)
            t_sc = inbuf.tile([P, TB, D], F32)  # normalized+masked target
            t_sq = inbuf.tile([P, TB, D], F32)
            t_ss = normp.tile([P, TB], F32)
            tT = inbuf.tile([P, T], BF16)  # [D, t tokens] normalized+masked

            # --- input DMAs, all on the sync ring, most critical first.
            nc.sync.dma_start(out=t_nat[:, 0:G, :], in_=tgt_r[:, 0:G, :])
            nc.sync.dma_start(out=mask_sb, in_=maskf.rearrange("p k -> p k"))
            nc.sync.dma_start(out=t_nat[:, G : 2 * G, :], in_=tgt_r[:, G : 2 * G, :])
            nc.sync.dma_start(out=ident, in_=identd.rearrange("p q -> p q"))
            nc.sync.dma_start(out=s_nat[:, 0:G, :], in_=src_r[:, 0:G, :])
            nc.sync.dma_start(out=s_nat[:, G : 2 * G, :], in_=src_r[:, G : 2 * G, :])
            nc.sync.dma_start(out=t_nat[:, HB:TB, :], in_=tgt_r[:, HB:TB, :])
            nc.sync.dma_start(out=s_nat[:, HB:SB, :], in_=src_r[:, HB:SB, :])

            cast_idx = [0]

            def xpose(src_tile, dstT, g, nm):
                # 4 PE transposes of fp32 [P,P] blocks -> one PSUM bank,
                # then one copy downconverting to bf16 (alternating engine).
                ps = psum_t.tile([P, G * P], F32, tag="pst", name=f"ps_{nm}{g}")
                for j in range(G):
                    k = g * G + j
                    nc.tensor.transpose(
                        ps[:, j * P : (j + 1) * P], src_tile[:, k, :], ident
                    )
                dst = dstT[:, g * G * P : (g + 1) * G * P]
                if cast_idx[0] % 2 == 0:
                    nc.vector.tensor_copy(out=dst, in_=ps)
                else:
                    nc.scalar.copy(out=dst, in_=ps)
                cast_idx[0] += 1

            def t_group_fast(g):
                # critical-path group: fused square+reduce per block on DVE
                # (no ACT table dependency), then rsqrt+mask+prescale+xpose.
                blk = slice(g * G, (g + 1) * G)
                nc.vector.tensor_mul(
                    out=t_sq[:, blk, :], in0=t_nat[:, blk, :], in1=t_nat[:, blk, :]
                )
                nc.vector.reduce_sum(
                    out=t_ss[:, blk], in_=t_sq[:, blk, :], axis=mybir.AxisListType.X
                )
                t_rcp = normp.tile([P, G], F32, tag="trcp", name=f"trcp{g}")
                nc.vector.reciprocal(out=t_rcp, in_=t_ss[:, blk])
                t_rsq = normp.tile([P, G], F32, tag="trsq", name=f"trsq{g}")
                nc.scalar.activation(out=t_rsq, in_=t_rcp, func=AF.Sqrt)
                t_scl = normp.tile([P, G], F32, tag="tscl2", name=f"tscl2_{g}")
                nc.vector.tensor_mul(
                    out=t_scl,
                    in0=t_rsq,
                    in1=mask_sb[:, SB + g * G : SB + (g + 1) * G],
                )
                scl_b = t_scl.unsqueeze(2).broadcast_to([P, G, D])
                nc.vector.tensor_mul(
                    out=t_sc[:, blk, :], in0=t_nat[:, blk, :], in1=scl_b
                )
                xpose(t_sc, tT, g, "t")

            def t_norm_h1():
                # batched chain for target half 1 (blocks 8-15), off the
                # critical path: ACT square + DVE reduce + prescale.
                blk = slice(HB, TB)
                nc.scalar.activation(
                    out=t_sq[:, blk, :], in_=t_nat[:, blk, :], func=AF.Square
                )
                nc.vector.reduce_sum(
                    out=t_ss[:, blk], in_=t_sq[:, blk, :], axis=mybir.AxisListType.X
                )
                t_rcp = normp.tile([P, HB], F32, tag="trcph", name="trcph1")
                nc.vector.reciprocal(out=t_rcp, in_=t_ss[:, blk])
                t_rsq = normp.tile([P, HB], F32, tag="trsqh", name="trsqh1")
                nc.scalar.activation(out=t_rsq, in_=t_rcp, func=AF.Sqrt)
                t_scl = normp.tile([P, HB], F32, tag="tsclh", name="tsclh1")
                nc.vector.tensor_mul(
                    out=t_scl, in0=t_rsq, in1=mask_sb[:, SB + HB : SB + TB]
                )
                scl_b = t_scl.unsqueeze(2).broadcast_to([P, HB, D])
                nc.vector.tensor_mul(
                    out=t_sc[:, blk, :], in0=t_nat[:, blk, :], in1=scl_b
                )

            def s_norm(h):
                # batched norm chain for source half h (8 blocks).
                blk = slice(h * HB, (h + 1) * HB)
                nc.scalar.activation(
                    out=s_sq[:, blk, :], in_=s_nat[:, blk, :], func=AF.Square
                )
                s_ss = normp.tile([P, HB], F32, tag="sss", name=f"sss{h}")
                nc.vector.reduce_sum(
                    out=s_ss, in_=s_sq[:, blk, :], axis=mybir.AxisListType.X
                )
                s_rcp = normp.tile([P, HB], F32, tag="srcp", name=f"srcp{h}")
                nc.vector.reciprocal(out=s_rcp, in_=s_ss)
                s_rsq = normp.tile([P, HB], F32, tag="srsq", name=f"srsq{h}")
                nc.scalar.activation(out=s_rsq, in_=s_rcp, func=AF.Sqrt)
                nc.vector.tensor_mul(
                    out=s_scl[:, blk],
                    in0=s_rsq,
                    in1=mask_sb[:, h * HB : (h + 1) * HB],
                )

            # --- main: 2 MMs -> 1024-wide fused scale+relu copy -> fp16.
            # Row-pairs (2q, 2q+1) share one 512KB DMA; the two copies of a
            # pair go to different engines so they run concurrently.
            copy_idx = [0]

            def half_row(m, h, ob_j):
                ps = psum_mm.tile([P, 2 * NT], F32, tag="psmm", name=f"mm{m}_{h}")
                for qq in range(2):
                    n = 2 * h + qq
                    nc.tensor.matmul(
                        ps[:, qq * NT : (qq + 1) * NT],
                        sT[:, m * P : (m + 1) * P],
                        tT[:, n * NT : (n + 1) * NT],
                        start=True,
                        stop=True,
                    )
                i = copy_idx[0]
                copy_idx[0] += 1
                if i % 2 == 0:
                    nc.scalar.activation(
                        out=ob_j, in_=ps, func=AF.Relu, scale=s_scl[:, m : m + 1]
                    )
                else:
                    nc.vector.tensor_scalar(
                        out=ob_j,
                        in0=ps,
                        scalar1=s_scl[:, m : m + 1],
                        scalar2=0.0,
                        op0=ALU.mult,
                        op1=ALU.max,
                    )

            def pair(q, h):
                ob = outp.tile([P, 2, 2 * NT], F16, tag="ob", name=f"ob{q}_{h}")
                half_row(2 * q, h, ob[:, 0, :])
                half_row(2 * q + 1, h, ob[:, 1, :])
                nc.sync.dma_start(
                    out=out_q[q][:, :, h * 2 * NT : (h + 1) * 2 * NT], in_=ob
                )

            # Emission order == scheduler priority. t groups 0/1 are the
            # critical path to the first output pairs -> pinned first.
            with tc.high_priority():
                t_group_fast(0)
                t_group_fast(1)
            xpose(s_nat, sT, 0, "s")
            xpose(s_nat, sT, 1, "s")
            s_norm(0)
            pair(0, 0)
            s_norm(1)
            pair(1, 0)
            t_norm_h1()
            pair(2, 0)
            xpose(s_nat, sT, 2, "s")
            pair(3, 0)
            xpose(s_nat, sT, 3, "s")
            pair(4, 0)
            xpose(t_sc, tT, 2, "t")
            pair(5, 0)
            xpose(t_sc, tT, 3, "t")
            pair(6, 0)
            pair(7, 0)
            for q in range(SB // 2):
                pair(q, 1)

    nc.compile()
    return nc


_NC_CACHE = None


def _get_nc():
    global _NC_CACHE
    if _NC_CACHE is None:
        _NC_CACHE = build_nc()
    return _NC_CACHE


_IDENT = np.eye(P, dtype=np.float32)


def kernel(source, target, mask_src, mask_tar, **run_kwargs):
    source = np.asarray(source, dtype=np.float32)
    target = np.asarray(target, dtype=np.float32)
    mask_src = np.asarray(mask_src)
    mask_tar = np.asarray(mask_tar)
    B = source.shape[0]

    in_maps = []
    for b in range(B):
        # source tokens in (p k) order; target tokens in (k p) order.
        msf = mask_src[b].astype(np.float32).reshape(P, SB)
        mtf = mask_tar[b].astype(np.float32).reshape(TB, P).T
        mk = np.ascontiguousarray(np.concatenate([msf, mtf], axis=1))
        in_maps.append(
            {
                "src": np.ascontiguousarray(source[b]),
                "tgt": np.ascontiguousarray(target[b]),
                "maskf": mk,
                "identd": _IDENT,
            }
        )

    nc = _get_nc()
    res = run_bass_kernel_spmd(nc, in_maps, core_ids=list(range(B)), **run_kwargs)
    out = np.stack(
        [np.asarray(r["out"], dtype=np.float32) for r in res.results], axis=0
    )
    if run_kwargs.get("trace"):
        kernel.last_results = res
    return out
